# revision 19
# baseline (speedup 1.0000x reference)
"""Trainium2 Bass kernel for nn_Dual_44100724196042 (gnn_message_passing).

Self-contained: host-side sharding/prep + 8-core SPMD Bass kernel + host
reduction of the per-core partial losses.

Strategy (row-shard n_node across 8 cores, 1000 rows each):
  - dense-ify the two edge lists on host into transposed adjacency matrices
    (G.T, M.T in bf16) so every spmm becomes a dense TensorE matmul with the
    adjacency streamed as the *moving* operand (outputs produced transposed,
    [128, 1000] per core)
  - 2 prop layers x 2 adjacencies + the feature@W embedding matmul, with
    AllGathers of the small [8000,128/256] row-major intermediates between
    layers (PE-transpose + bounce through DRAM)
  - gate attention: per-core partial w-sums -> tiny AllReduce -> softmax ->
    weighted sum, all in transposed layout
  - losses: one-hot/count-matrix tricks turn every gather into a dense
    matmul; the [8000,8000] contrastive score matrix is computed in
    [125,500] tiles (matmul -> fused exp+rowsum on ScalarE -> fused
    pos-mask-mul+reduce on VectorE) and never materialized
  - per-core partial losses returned as [128,16] f32; host sums.
"""

import os
import sys
import types
import numpy as np

NCORES = 8
N_USER, N_ITEM, N_NODE = 3000, 5000, 8000
D, E, B, L = 64, 262144, 1024, 50
TAU, NEG_W, PR_W, CON_W = 0.2, 0.1, 1.0, 1e-3
RPC = N_NODE // NCORES      # 1000 rows per core
BPC = B // NCORES           # 128 batch rows per core
KT = 125                    # contraction tile (8000 = 64*125)
NKT = N_NODE // KT          # 64
CW = 500                    # free-dim chunk width for phase B
OUT_COLS = 16               # per-core output [128, 16]
# output column slots
C_CON, C_A, C_B, C_PR, C_AD = 0, 1, 2, 3, 4


# --------------------------------------------------------------------------
# Tile drain workaround: walrus in this container rejects the TileContext
# exit drain when it carries >2 sem waits ("Too many sync wait commands").
# Split the waits across single-wait sync-engine nops; SP program order makes
# the cumulative wait equivalent, so the drain itself needs none.
# --------------------------------------------------------------------------
_PATCHED = False


def _apply_tile_patch():
    global _PATCHED
    if _PATCHED:
        return
    import bass_rust
    import concourse.tile as tile
    from concourse.tile import ScopedClock

    def _split_drain_and_barrier(self, tick_clock, wait_clock):
        gc = tick_clock.global_clock
        s = str(gc)
        inner = s[s.index('[') + 1:s.index(']')]
        vals = [int(x) for x in inner.split(',')] if inner.strip() else []
        for i, v in enumerate(vals):
            if v > 0:
                single = [0] * len(vals)
                single[i] = v
                nop = self.nc.sync.nop(nofuse=True)
                wait_clock.add_sem_waits(
                    nop.ins, ScopedClock({None: bass_rust.VectorClock(single)})
                )
        self.nc.sync.drain()
        self.nc.all_engine_barrier()
        assert self.sems is not None
        popped = self.nc._tile_sem_poison_stack.pop()
        assert popped is self._sem_poison
        self.nc.clear_and_free_semaphores(list(self.sems.allocated().values()))
        self.nc.all_engine_barrier()

    tile.TileContext._drain_and_barrier = _split_drain_and_barrier
    _PATCHED = True


def _split_sync_waits(nc, maxw=1):
    """This container's walrus rejects instructions carrying more than ~2 sem
    waits ("Too many sync wait commands"). Move excess waits onto injected
    same-engine nops immediately before the instruction — engine streams are
    in-order, so the cumulative gating is identical."""
    import bass_rust

    blocks = list(nc.main_func.blocks)
    with nc.semaphore("waitsplit_dummy") as dummy:
        for bb in blocks:
            il = bb.instructions
            idx = 0
            while idx < len(il):
                ins = il[idx]
                si = ins.sync_info
                if si is None or not si.on_wait or len(si.on_wait) <= maxw:
                    idx += 1
                    continue
                waits = list(si.on_wait)
                excess, keep = waits[:-maxw], waits[-maxw:]
                si.on_wait = keep
                eng = ins.engine
                nops = []
                for j in range(0, len(excess), maxw):
                    nb = nc.engines[eng].nop(nofuse=True)
                    nin = nb.ins
                    src_lst = nc.cur_bb.bb.instructions
                    for k in range(len(src_lst) - 1, -1, -1):
                        if src_lst[k].name == nin.name:
                            del src_lst[k]
                            break
                    bass_rust.wait_op(nin, dummy, 1, "sem-ge", True)
                    nin.sync_info.on_wait = excess[j:j + maxw]
                    nops.append(nin)
                for n_i, nin in enumerate(nops):
                    il.insert(idx + n_i, nin)
                idx += len(nops) + 1


# --------------------------------------------------------------------------
# kernel builder
# --------------------------------------------------------------------------
def build_nc():
    _apply_tile_patch()
    STAGE = int(os.environ.get("K_STAGE", "99"))
    import concourse.bass as bass
    import concourse.tile as tile
    from concourse import mybir
    from concourse.bass import ts
    from concourse.masks import make_identity
    from contextlib import ExitStack

    BF = mybir.dt.bfloat16
    F32 = mybir.dt.float32
    AX = mybir.AxisListType.X
    AF = mybir.ActivationFunctionType
    OP = mybir.AluOpType
    RG = [list(range(NCORES))]

    nc = bass.Bass(num_devices=NCORES)

    # ---- kernel I/O ----
    featT = nc.declare_dram_parameter("featT", [KT, NKT, RPC], BF, isOutput=False)
    gT = nc.declare_dram_parameter("gT", [KT, NKT, RPC], BF, isOutput=False)
    mT = nc.declare_dram_parameter("mT", [KT, NKT, RPC], BF, isOutput=False)
    w12 = nc.declare_dram_parameter("w12", [N_NODE, 128], BF, isOutput=False)
    pos = nc.declare_dram_parameter("pos", [RPC, N_NODE], BF, isOutput=False)
    scT = nc.declare_dram_parameter("scT", [KT, N_USER // KT, BPC], BF, isOutput=False)
    cc = nc.declare_dram_parameter("cc", [BPC, N_ITEM], BF, isOutput=False)
    prl = nc.declare_dram_parameter("prl", [BPC, N_ITEM], F32, isOutput=False)
    gw1T_re = nc.declare_dram_parameter("gw1T_re", [D, D], BF, isOutput=False)
    gw1T_pr = nc.declare_dram_parameter("gw1T_pr", [D, D], BF, isOutput=False)
    gb1_re = nc.declare_dram_parameter("gb1_re", [D, 1], F32, isOutput=False)
    gb1_pr = nc.declare_dram_parameter("gb1_pr", [D, 1], F32, isOutput=False)
    gw2_re = nc.declare_dram_parameter("gw2_re", [D, 1], BF, isOutput=False)
    gw2_pr = nc.declare_dram_parameter("gw2_pr", [D, 1], BF, isOutput=False)
    selscale = nc.declare_dram_parameter("selscale", [2, 1], F32, isOutput=False)
    sel01 = nc.declare_dram_parameter("sel01", [2, 1], F32, isOutput=False)
    rre_row = nc.declare_dram_parameter("rre_row", [1, D], F32, isOutput=False)
    rre_col = nc.declare_dram_parameter("rre_col", [D, 1], F32, isOutput=False)
    rpr_col = nc.declare_dram_parameter("rpr_col", [D, 1], F32, isOutput=False)
    out = nc.declare_dram_parameter("out", [128, OUT_COLS], F32, isOutput=True)

    def dma_eng(i):
        return nc.sync if i % 2 == 0 else nc.scalar

    def bcast(ap, parts):
        # DRAM source broadcast across partitions (step-0 partition dim)
        return bass.AP(tensor=ap.tensor, offset=ap.offset,
                       ap=[[0, parts]] + [list(d) for d in ap.ap[-1:]])

    with tile.TileContext(nc) as tc, ExitStack() as ctx:
        pc = ctx.enter_context(tc.tile_pool(name="pc", bufs=1))
        pdram = ctx.enter_context(tc.tile_pool(name="pdram", bufs=1, space="DRAM"))
        psmall = ctx.enter_context(tc.tile_pool(name="psmall", bufs=1, space="PSUM"))

        # ---- constants & small params ----
        ident = pc.tile([128, 128], BF)
        make_identity(nc, ident)
        ones64 = pc.tile([D, 1], F32)
        nc.vector.memset(ones64, 1.0)
        ones2 = pc.tile([2, 1], F32)
        nc.vector.memset(ones2, 1.0)

        def load(shape, dt, src, tag):
            t = pc.tile(shape, dt, tag=tag)
            nc.sync.dma_start(out=t, in_=src)
            return t

        gw1T_re_s = load([D, D], BF, gw1T_re[:, :], "gw1T_re_s")
        gw1T_pr_s = load([D, D], BF, gw1T_pr[:, :], "gw1T_pr_s")
        gb1_re_s = load([D, 1], F32, gb1_re[:, :], "gb1_re_s")
        gb1_pr_s = load([D, 1], F32, gb1_pr[:, :], "gb1_pr_s")
        gw2_re_s = load([D, 1], BF, gw2_re[:, :], "gw2_re_s")
        gw2_pr_s = load([D, 1], BF, gw2_pr[:, :], "gw2_pr_s")
        selscale_s = load([2, 1], F32, selscale[:, :], "selscale_s")
        sel01_s = load([2, 1], F32, sel01[:, :], "sel01_s")
        rre_row_s = load([1, D], F32, rre_row[:, :], "rre_row_s")
        rre_col_s = load([D, 1], F32, rre_col[:, :], "rre_col_s")
        rpr_col_s = load([D, 1], F32, rpr_col[:, :], "rpr_col_s")

        # persistent SBUF intermediates
        i12_sb = pc.tile([128, RPC], BF)     # [i1;i2].T
        i34_sb = pc.tile([128, RPC], BF)     # [i3;i4].T
        # base-partition-0 copies of the upper halves (PE/DVE operands must
        # share a base partition; only DMA can shift partitions)
        i2_sb = pc.tile([D, RPC], BF)
        i4_sb = pc.tile([D, RPC], BF)
        gre_sb = pc.tile([D, RPC], BF)       # gate output (re), transposed
        gpr_sb = pc.tile([D, RPC], BF)       # gate output (pr), transposed
        w6 = pc.tile([1, 6], F32)
        out_sb = pc.tile([128, OUT_COLS], F32)
        nc.vector.memset(out_sb, 0.0)

        # DRAM bounces / collective buffers
        e_rm = pdram.tile([RPC, 128], BF)
        y_rm = pdram.tile([RPC, 256], BF)
        go_rm = pdram.tile([RPC, 128], BF)
        go_tr = pdram.tile([128, RPC], BF)
        E_ag = pdram.tile([N_NODE, 128], BF)
        Y_ag = pdram.tile([N_NODE, 256], BF)
        GRM_ag = pdram.tile([N_NODE, 128], BF)
        GTR_ag = pdram.tile([128 * NCORES, RPC], BF)
        ar_in = pdram.tile([2, 6], F32)
        ar_out = pdram.tile([2, 6], F32)
        s6d = pdram.tile([1, 6], F32)
        betad = pdram.tile([1, 6], F32)
        n2prd = pdram.tile([N_NODE], F32)
        invprd = pdram.tile([N_NODE], BF)
        n2red = pdram.tile([RPC], F32)

        def transpose_to_dram(src_sb, dst_dram, dst_col0, psT, pstage):
            # src_sb [128, RPC] bf16 -> dst_dram rows [RPC, ...] cols dst_col0:+128
            for t in range(RPC // KT):
                tp = psT.tile([KT, 128], BF, tag="tp")
                nc.tensor.transpose(tp, src_sb[:, ts(t, KT)], ident)
                st = pstage.tile([KT, 128], BF, tag="st")
                nc.vector.tensor_copy(st, tp)
                nc.sync.dma_start(
                    out=dst_dram[ts(t, KT), dst_col0:dst_col0 + 128], in_=st)

        # ================= PHASE A =================
        with (
            tc.tile_pool(name="pA", bufs=1) as pA,
            tc.tile_pool(name="pmov", bufs=3) as pmov,
            tc.tile_pool(name="psA", bufs=2, space="PSUM") as psA,
            tc.tile_pool(name="psT", bufs=2, space="PSUM") as psT,
            tc.tile_pool(name="pstage", bufs=3) as pstage,
        ):
            KG = 8          # k-tiles per moving DMA (16KB/partition contiguous)
            NG = NKT // KG  # 8 groups
            with nc.named_scope("A_feat"):
                W_sb = pA.tile([KT, NKT, 128], BF)
                nc.sync.dma_start(
                    out=W_sb, in_=w12[:, :].rearrange("(p t) c -> p t c", p=KT))
                ps_e = psA.tile([128, 1024], F32, tag="acc")
                for g in range(NG):
                    mv = pmov.tile([KT, KG, RPC], BF, tag="mvg")
                    dma_eng(g).dma_start(out=mv, in_=featT[:, ts(g, KG), :])
                    for kk in range(KG):
                        k = g * KG + kk
                        nc.tensor.matmul(ps_e[:, 0:512], W_sb[:, k, :],
                                         mv[:, kk, 0:512],
                                         start=(k == 0), stop=(k == NKT - 1))
                        nc.tensor.matmul(ps_e[:, 512:RPC], W_sb[:, k, :],
                                         mv[:, kk, 512:RPC],
                                         start=(k == 0), stop=(k == NKT - 1))
                ecT = pA.tile([128, RPC], BF)
                nc.vector.tensor_copy(ecT, ps_e[:, 0:RPC])
                transpose_to_dram(ecT, e_rm, 0, psT, pstage)
            nc.gpsimd.collective_compute(
                "AllGather", mybir.AluOpType.bypass,
                ins=[e_rm.opt()], outs=[E_ag.opt()], replica_groups=RG)

            with nc.named_scope("A_layer1"):
                E_sb = pA.tile([KT, NKT, 128], BF)
                nc.sync.dma_start(
                    out=E_sb, in_=E_ag[:, :].rearrange("(p t) c -> p t c", p=KT))
                ps_g = psA.tile([128, 1024], F32, tag="acc")
                ps_m = psA.tile([128, 1024], F32, tag="acc")
                for g in range(NG):
                    mvg = pmov.tile([KT, KG, RPC], BF, tag="mvg")
                    dma_eng(g).dma_start(out=mvg, in_=gT[:, ts(g, KG), :])
                    mvm = pmov.tile([KT, KG, RPC], BF, tag="mvm")
                    dma_eng(g + 1).dma_start(out=mvm, in_=mT[:, ts(g, KG), :])
                    for kk in range(KG):
                        k = g * KG + kk
                        st, sp = (k == 0), (k == NKT - 1)
                        nc.tensor.matmul(ps_g[:, 0:512], E_sb[:, k, :],
                                         mvg[:, kk, 0:512], start=st, stop=sp)
                        nc.tensor.matmul(ps_g[:, 512:RPC], E_sb[:, k, :],
                                         mvg[:, kk, 512:RPC], start=st, stop=sp)
                        # M side wants stationary [e2|e1]: two half-matmuls
                        nc.tensor.matmul(ps_m[0:64, 0:512], E_sb[:, k, 64:128],
                                         mvm[:, kk, 0:512], start=st, stop=sp)
                        nc.tensor.matmul(ps_m[0:64, 512:RPC], E_sb[:, k, 64:128],
                                         mvm[:, kk, 512:RPC], start=st, stop=sp)
                        nc.tensor.matmul(ps_m[64:128, 0:512], E_sb[:, k, 0:64],
                                         mvm[:, kk, 0:512], start=st, stop=sp)
                        nc.tensor.matmul(ps_m[64:128, 512:RPC], E_sb[:, k, 0:64],
                                         mvm[:, kk, 512:RPC], start=st, stop=sp)
                ygT = pA.tile([128, RPC], BF)
                nc.vector.tensor_copy(ygT, ps_g[:, 0:RPC])
                ymT = pA.tile([128, RPC], BF)
                nc.vector.tensor_copy(ymT, ps_m[:, 0:RPC])
                transpose_to_dram(ygT, y_rm, 0, psT, pstage)
                transpose_to_dram(ymT, y_rm, 128, psT, pstage)
            nc.gpsimd.collective_compute(
                "AllGather", mybir.AluOpType.bypass,
                ins=[y_rm.opt()], outs=[Y_ag.opt()], replica_groups=RG)

            with nc.named_scope("A_layer2"):
                Y_sb = pA.tile([KT, NKT, 256], BF)
                nc.sync.dma_start(
                    out=Y_sb, in_=Y_ag[:, :].rearrange("(p t) c -> p t c", p=KT))
                ps_i12 = psA.tile([128, 1024], F32, tag="acc")
                ps_i34 = psA.tile([128, 1024], F32, tag="acc")
                for g in range(NG):
                    mvg = pmov.tile([KT, KG, RPC], BF, tag="mvg")
                    dma_eng(g).dma_start(out=mvg, in_=gT[:, ts(g, KG), :])
                    mvm = pmov.tile([KT, KG, RPC], BF, tag="mvm")
                    dma_eng(g + 1).dma_start(out=mvm, in_=mT[:, ts(g, KG), :])
                    for kk in range(KG):
                        k = g * KG + kk
                        st, sp = (k == 0), (k == NKT - 1)
                        nc.tensor.matmul(ps_i12[:, 0:512], Y_sb[:, k, 0:128],
                                         mvg[:, kk, 0:512], start=st, stop=sp)
                        nc.tensor.matmul(ps_i12[:, 512:RPC], Y_sb[:, k, 0:128],
                                         mvg[:, kk, 512:RPC], start=st, stop=sp)
                        nc.tensor.matmul(ps_i34[:, 0:512], Y_sb[:, k, 128:256],
                                         mvm[:, kk, 0:512], start=st, stop=sp)
                        nc.tensor.matmul(ps_i34[:, 512:RPC], Y_sb[:, k, 128:256],
                                         mvm[:, kk, 512:RPC], start=st, stop=sp)
                nc.vector.tensor_copy(i12_sb, ps_i12[:, 0:RPC])
                nc.vector.tensor_copy(i34_sb, ps_i34[:, 0:RPC])
                nc.sync.dma_start(out=i2_sb, in_=i12_sb[64:128, :])
                nc.sync.dma_start(out=i4_sb, in_=i34_sb[64:128, :])

        # ================= GATES =================
        if STAGE >= 2:
         with (
            nc.named_scope("gates"),
            tc.tile_pool(name="psG", bufs=1, space="PSUM") as psG,
            tc.tile_pool(name="pg", bufs=2) as pg,
        ):
            # z planes: re -> (i1,i2,i3); pr -> (i2,i3,i4)
            zplanes = {
                0: (i12_sb[0:64, :], i2_sb[:, :], i34_sb[0:64, :]),
                1: (i2_sb[:, :], i34_sb[0:64, :], i4_sb[:, :]),
            }
            gparams = {0: (gw1T_re_s, gb1_re_s, gw2_re_s),
                       1: (gw1T_pr_s, gb1_pr_s, gw2_pr_s)}
            for gi in (0, 1):
                w1T_s, b1_s, w2_s = gparams[gi]
                for s in range(3):
                    zT = zplanes[gi][s]
                    ps_h = psG.tile([D, 1024], F32, tag="h")
                    nc.tensor.matmul(ps_h[:, 0:512], w1T_s, zT[:, 0:512])
                    nc.tensor.matmul(ps_h[:, 512:RPC], w1T_s, zT[:, 512:RPC])
                    h_sb = pg.tile([D, RPC], BF, tag="h_sb")
                    nc.scalar.activation(h_sb, ps_h[:, 0:RPC], AF.Tanh, bias=b1_s)
                    ps_w = psG.tile([1, 1024], F32, tag="w")
                    nc.tensor.matmul(ps_w[:, 0:512], w2_s, h_sb[:, 0:512])
                    nc.tensor.matmul(ps_w[:, 512:RPC], w2_s, h_sb[:, 512:RPC])
                    nc.vector.tensor_reduce(
                        w6[0:1, gi * 3 + s:gi * 3 + s + 1], ps_w[0:1, 0:RPC],
                        AX, OP.add)
            # mask+scale partials, AllReduce, softmax per group
            nc.sync.dma_start(out=s6d, in_=w6)
            w6b = pg.tile([2, 6], F32, tag="w6b")
            nc.sync.dma_start(out=w6b, in_=bcast(s6d[:, :], 2))
            ar_sb = pg.tile([2, 6], F32, tag="ar_sb")
            nc.vector.tensor_scalar_mul(ar_sb, w6b, selscale_s)
            nc.sync.dma_start(out=ar_in, in_=ar_sb)
            nc.gpsimd.collective_compute(
                "AllReduce", mybir.AluOpType.add,
                ins=[ar_in.opt()], outs=[ar_out.opt()], replica_groups=RG)
            aro = pg.tile([2, 6], F32, tag="aro")
            nc.sync.dma_start(out=aro, in_=ar_out)
            bm = pg.tile([2, 6], F32, tag="bm")
            for h0 in (0, 3):
                m0 = pg.tile([2, 1], F32, tag="m0")
                nc.vector.tensor_reduce(m0, aro[:, h0:h0 + 3], AX, OP.max)
                negm0 = pg.tile([2, 1], F32, tag="negm0")
                nc.vector.tensor_scalar_mul(negm0, m0, -1.0)
                e0 = pg.tile([2, 3], F32, tag="e0")
                nc.scalar.activation(e0, aro[:, h0:h0 + 3], AF.Exp, bias=negm0)
                s0 = pg.tile([2, 1], F32, tag="s0")
                nc.vector.tensor_reduce(s0, e0, AX, OP.add)
                r0 = pg.tile([2, 1], F32, tag="r0")
                nc.vector.reciprocal(r0, s0)
                nc.vector.tensor_scalar(
                    bm[:, h0:h0 + 3], e0, r0, sel01_s, OP.mult, OP.mult)
            ps_b6 = psmall.tile([6, 1], F32, tag="b6")
            nc.tensor.matmul(ps_b6, bm, ones2)
            b6 = pg.tile([6, 1], F32, tag="b6s")
            nc.vector.tensor_copy(b6, ps_b6)
            nc.sync.dma_start(out=betad, in_=b6[:, 0])
            beta_b = pc.tile([D, 6], F32)
            nc.sync.dma_start(out=beta_b, in_=bcast(betad[:, :], D))
            # gate outputs (weighted sums)
            for gi, gout in ((0, gre_sb), (1, gpr_sb)):
                z0, z1, z2 = zplanes[gi]
                t1 = pg.tile([D, RPC], F32, tag="t1")
                nc.vector.tensor_scalar_mul(t1, z0, beta_b[:, 3 * gi:3 * gi + 1])
                t2 = pg.tile([D, RPC], F32, tag="t2")
                nc.vector.scalar_tensor_tensor(
                    t2, z1, beta_b[:, 3 * gi + 1:3 * gi + 2], t1, OP.mult, OP.add)
                nc.vector.scalar_tensor_tensor(
                    gout, z2, beta_b[:, 3 * gi + 2:3 * gi + 3], t2,
                    OP.mult, OP.add)
            # bounce both layouts + AGs
            nc.sync.dma_start(out=go_tr[0:64, :], in_=gre_sb)
            nc.sync.dma_start(out=go_tr[64:128, :], in_=gpr_sb)
            with (tc.tile_pool(name="psT2", bufs=2, space="PSUM") as psT2,
                  tc.tile_pool(name="pst2", bufs=3) as pst2):
                for gi, gout in ((0, gre_sb), (1, gpr_sb)):
                    for t in range(RPC // KT):
                        tp = psT2.tile([KT, D], BF, tag="tp")
                        nc.tensor.transpose(tp, gout[:, ts(t, KT)],
                                            ident[0:64, 0:64])
                        st = pst2.tile([KT, D], BF, tag="st")
                        nc.vector.tensor_copy(st, tp)
                        nc.sync.dma_start(
                            out=go_rm[ts(t, KT), 64 * gi:64 * gi + 64], in_=st)
        nc.gpsimd.collective_compute(
            "AllGather", mybir.AluOpType.bypass,
            ins=[go_tr.opt()], outs=[GTR_ag.opt()], replica_groups=RG)
        nc.gpsimd.collective_compute(
            "AllGather", mybir.AluOpType.bypass,
            ins=[go_rm.opt()], outs=[GRM_ag.opt()], replica_groups=RG)

        # ================= PHASE B =================
        if STAGE >= 3:
         with (
            tc.tile_pool(name="pB", bufs=1) as pB,
            tc.tile_pool(name="ppos", bufs=2) as ppos,
            tc.tile_pool(name="psS", bufs=3, space="PSUM") as psS,
            tc.tile_pool(name="psB", bufs=1, space="PSUM") as psB,
            tc.tile_pool(name="pj", bufs=2) as pj,
            tc.tile_pool(name="pacc", bufs=2) as pacc,
        ):
            with nc.named_scope("B_norm"):
                embprT = pB.tile([D, N_NODE], BF)
                for c in range(NCORES):
                    nc.sync.dma_start(
                        out=embprT[:, c * RPC:(c + 1) * RPC],
                        in_=GTR_ag[c * 128 + 64:c * 128 + 128, :])
                HiT = pB.tile([D, N_ITEM], BF)
                for c in range(3, NCORES):
                    nc.sync.dma_start(
                        out=HiT[:, (c - 3) * RPC:(c - 2) * RPC],
                        in_=GTR_ag[c * 128:c * 128 + 64, :])
                # emb_pr column norms -> 1/sqrt -> scaled copy
                for nt in range(N_NODE // CW):
                    sq = pj.tile([D, CW], F32, tag="sq")
                    nc.vector.tensor_mul(sq, embprT[:, ts(nt, CW)],
                                         embprT[:, ts(nt, CW)])
                    psn = psB.tile([1, 512], F32, tag="n")
                    nc.tensor.matmul(psn[0:1, 0:CW], ones64, sq)
                    stg = pj.tile([1, 512], F32, tag="stg")
                    if nt % 2 == 0:
                        nc.scalar.copy(stg[0:1, 0:CW], psn[0:1, 0:CW])
                    else:
                        nc.vector.tensor_copy(stg[0:1, 0:CW], psn[0:1, 0:CW])
                    nc.sync.dma_start(out=n2prd[ts(nt, CW)], in_=stg[0:1, 0:CW])
                n2rs = pB.tile([KT, NKT], F32)
                nc.sync.dma_start(
                    out=n2rs, in_=n2prd[:].rearrange("(j p) -> p j", p=KT))
                nc.vector.reciprocal(n2rs, n2rs)
                inv_rs = pB.tile([KT, NKT], F32)
                nc.scalar.activation(inv_rs, n2rs, AF.Sqrt)
                inv_bf = pB.tile([KT, NKT], BF)
                nc.vector.tensor_copy(inv_bf, inv_rs)
                nc.sync.dma_start(
                    out=invprd[:].rearrange("(j p) -> p j", p=KT), in_=inv_bf)
                invpr_b = pB.tile([D, N_NODE], BF)
                nc.sync.dma_start(out=invpr_b, in_=bcast(invprd[None, :], D))
                embprS = pB.tile([D, N_NODE], BF)
                nc.vector.tensor_mul(embprS, embprT, invpr_b)
                # own emb_re row norms -> invre/tau in [125, 8] layout
                for nt in range(RPC // CW):
                    sqr = pj.tile([D, CW], F32, tag="sq")
                    nc.vector.tensor_mul(sqr, gre_sb[:, ts(nt, CW)],
                                         gre_sb[:, ts(nt, CW)])
                    psn = psB.tile([1, 512], F32, tag="n")
                    nc.tensor.matmul(psn[0:1, 0:CW], ones64, sqr)
                    stg = pj.tile([1, 512], F32, tag="stg")
                    nc.vector.tensor_copy(stg[0:1, 0:CW], psn[0:1, 0:CW])
                    nc.sync.dma_start(out=n2red[ts(nt, CW)], in_=stg[0:1, 0:CW])
                n2re_rs = pB.tile([KT, RPC // KT], F32)
                nc.sync.dma_start(
                    out=n2re_rs, in_=n2red[:].rearrange("(j p) -> p j", p=KT))
                nc.vector.reciprocal(n2re_rs, n2re_rs)
                invre_s = pB.tile([KT, RPC // KT], F32)
                nc.scalar.activation(invre_s, n2re_rs, AF.Sqrt)
                invre_tau = pB.tile([KT, RPC // KT], F32)
                nc.vector.tensor_scalar_mul(invre_tau, invre_s, 1.0 / TAU)

            if STAGE >= 4:
             with nc.named_scope("B_con"):
                con_acc = pB.tile([KT, RPC // KT], F32)
                for mt in range(RPC // KT):
                    posr = ppos.tile([KT, N_NODE], BF, tag="pos")
                    dma_eng(mt).dma_start(out=posr, in_=pos[ts(mt, KT), :])
                    rsum_acc = pacc.tile([KT, 16], F32, tag="rs")
                    psum_acc = pacc.tile([KT, 16], F32, tag="pssc")
                    for nt in range(N_NODE // CW):
                        pss = psS.tile([128, 512], F32, tag="chunk")
                        nc.tensor.matmul(pss[0:KT, 0:CW],
                                         gre_sb[:, ts(mt, KT)],
                                         embprS[:, ts(nt, CW)])
                        s_sb = pj.tile([KT, CW], BF, tag="s_sb")
                        nc.scalar.activation(
                            s_sb, pss[0:KT, 0:CW], AF.Exp,
                            scale=invre_tau[:, mt:mt + 1],
                            accum_out=rsum_acc[:, nt:nt + 1])
                        jk = pj.tile([KT, CW], BF, tag="jk")
                        nc.vector.scalar_tensor_tensor(
                            jk, s_sb, 1.0, posr[:, ts(nt, CW)],
                            OP.mult, OP.mult,
                            accum_out=psum_acc[:, nt:nt + 1])
                    rs1 = pacc.tile([KT, 1], F32, tag="rs1")
                    nc.vector.tensor_reduce(rs1, rsum_acc, AX, OP.add)
                    ps1 = pacc.tile([KT, 1], F32, tag="ps1")
                    nc.vector.tensor_reduce(ps1, psum_acc, AX, OP.add)
                    neg = pacc.tile([KT, 1], F32, tag="neg")
                    nc.vector.tensor_sub(neg, rs1, ps1)
                    ln_n = pacc.tile([KT, 1], F32, tag="ln_n")
                    nc.scalar.activation(ln_n, neg, AF.Ln)
                    ln_p = pacc.tile([KT, 1], F32, tag="ln_p")
                    nc.scalar.activation(ln_p, ps1, AF.Ln)
                    nc.vector.tensor_sub(con_acc[:, mt:mt + 1], ln_n, ln_p)
                nc.vector.tensor_reduce(
                    out_sb[0:KT, C_CON:C_CON + 1], con_acc, AX, OP.add)

            if STAGE >= 5:
             with nc.named_scope("B_rec"):
                # h_u gather via one-hot matmul: [128b x 128d]
                scT_sb = pB.tile([KT, N_USER // KT, BPC], BF)
                nc.sync.dma_start(out=scT_sb, in_=scT[:, :, :])
                agu = pB.tile([KT, N_USER // KT, 128], BF)
                nc.scalar.dma_start(
                    out=agu,
                    in_=GRM_ag[0:N_USER, :].rearrange("(p t) c -> p t c", p=KT))
                ps_hu = psS.tile([128, 512], F32, tag="chunk")
                nku = N_USER // KT
                for k in range(nku):
                    nc.tensor.matmul(ps_hu[:, 0:128], scT_sb[:, k, :], agu[:, k, :],
                                     start=(k == 0), stop=(k == nku - 1))
                hu_sb = pB.tile([BPC, 128], F32)
                nc.vector.tensor_copy(hu_sb, ps_hu[:, 0:128])
                hu_bf = pB.tile([BPC, 128], BF)
                nc.vector.tensor_copy(hu_bf, ps_hu[:, 0:128])
                # part1 = hu_re.T @ hu_re  (fp32)
                ps_p1 = psB.tile([D, D], F32, tag="p")
                nc.tensor.matmul(ps_p1, hu_sb[:, 0:64], hu_sb[:, 0:64])
                p1_sb = pB.tile([D, D], F32)
                nc.vector.tensor_copy(p1_sb, ps_p1)
                # part2 = Hi.T @ Hi
                it64 = pB.tile([KT, N_ITEM // KT, 128], BF)
                nc.sync.dma_start(
                    out=it64,
                    in_=GRM_ag[N_USER:N_NODE, :].rearrange(
                        "(p t) c -> p t c", p=KT))
                ps_p2 = psB.tile([D, D], F32, tag="p")
                nki = N_ITEM // KT
                for k in range(nki):
                    nc.tensor.matmul(ps_p2, it64[:, k, 0:64], it64[:, k, 0:64],
                                     start=(k == 0), stop=(k == nki - 1))
                p2_sb = pB.tile([D, D], F32)
                nc.vector.tensor_copy(p2_sb, ps_p2)
                # part3 = r_re r_re.T
                ps_p3 = psB.tile([D, D], F32, tag="p")
                nc.tensor.matmul(ps_p3, rre_row_s, rre_row_s)
                p3_sb = pB.tile([D, D], F32)
                nc.vector.tensor_copy(p3_sb, ps_p3)
                t12 = pB.tile([D, D], F32)
                nc.vector.tensor_mul(t12, p1_sb, p2_sb)
                jk64 = pB.tile([D, D], F32)
                ad_col = pB.tile([D, 1], F32)
                nc.vector.scalar_tensor_tensor(
                    jk64, t12, 1.0, p3_sb, OP.mult, OP.mult, accum_out=ad_col)
                nc.vector.tensor_copy(out_sb[0:D, C_AD:C_AD + 1], ad_col)
                # qT = (hu_re * r_re).T ; qprT = (hu_pr * r_pr).T
                ps_qt = psB.tile([D, 128], BF, tag="qt")
                nc.tensor.transpose(ps_qt, hu_bf[:, 0:64], ident)
                qT_sb = pB.tile([D, BPC], BF)
                nc.scalar.activation(qT_sb, ps_qt, AF.Copy, bias=0.0,
                                     scale=rre_col_s)
                ps_qpt = psB.tile([D, 128], BF, tag="qt")
                nc.tensor.transpose(ps_qpt, hu_bf[:, 64:128], ident)
                qprT_sb = pB.tile([D, BPC], BF)
                nc.scalar.activation(qprT_sb, ps_qpt, AF.Copy, bias=0.0,
                                     scale=rpr_col_s)
                # hpq loops
                a_acc = pacc.tile([BPC, N_ITEM // CW], F32, tag="a_acc")
                b_acc = pacc.tile([BPC, N_ITEM // CW], F32, tag="b_acc")
                pr_acc = pacc.tile([BPC, N_ITEM // CW], F32, tag="pr_acc")
                for nt in range(N_ITEM // CW):
                    ps_h1 = psS.tile([128, 512], F32, tag="chunk")
                    nc.tensor.matmul(ps_h1[:, 0:CW], qT_sb, HiT[:, ts(nt, CW)])
                    hq = pj.tile([BPC, CW], BF, tag="hq")
                    nc.scalar.activation(hq, ps_h1[:, 0:CW], AF.Copy)
                    ccb = pj.tile([BPC, CW], BF, tag="ccb")
                    nc.sync.dma_start(out=ccb, in_=cc[:, ts(nt, CW)])
                    u = pj.tile([BPC, CW], BF, tag="u")
                    nc.vector.tensor_mul(u, hq, ccb)
                    jk2 = pj.tile([BPC, CW], BF, tag="jk2")
                    nc.vector.scalar_tensor_tensor(
                        jk2, u, 1.0, hq, OP.mult, OP.mult,
                        accum_out=a_acc[:, nt:nt + 1])
                    nc.vector.tensor_reduce(b_acc[:, nt:nt + 1], u, AX, OP.add)
                    ps_h2 = psS.tile([128, 512], F32, tag="chunk")
                    nc.tensor.matmul(ps_h2[:, 0:CW], qprT_sb,
                                     embprT[:, N_USER + nt * CW:N_USER + (nt + 1) * CW])
                    prb = pj.tile([BPC, CW], F32, tag="prb")
                    nc.sync.dma_start(out=prb, in_=prl[:, ts(nt, CW)])
                    dti = pj.tile([BPC, CW], F32, tag="dti")
                    nc.vector.scalar_tensor_tensor(
                        dti, ps_h2[:, 0:CW], 1.0, prb,
                        OP.mult, OP.subtract)
                    jk3 = pj.tile([BPC, CW], F32, tag="jk3")
                    nc.vector.scalar_tensor_tensor(
                        jk3, dti, 1.0, dti, OP.mult, OP.mult,
                        accum_out=pr_acc[:, nt:nt + 1])
                nc.vector.tensor_reduce(out_sb[:, C_A:C_A + 1], a_acc, AX, OP.add)
                nc.vector.tensor_reduce(out_sb[:, C_B:C_B + 1], b_acc, AX, OP.add)
                nc.vector.tensor_reduce(out_sb[:, C_PR:C_PR + 1], pr_acc, AX, OP.add)

        nc.sync.dma_start(out=out[:, :], in_=out_sb)

    _split_sync_waits(nc)
    return nc


# --------------------------------------------------------------------------
# host-side prep
# --------------------------------------------------------------------------
def prepare_in_maps(inputs):
    import ml_dtypes
    bf16 = ml_dtypes.bfloat16
    f = {k: np.asarray(v) for k, v in inputs.items()}

    GT = np.zeros((N_NODE, N_NODE), np.float32)
    np.add.at(GT, (f["graph_col"], f["graph_row"]), f["graph_val"])
    GT = GT.astype(bf16)
    MT = np.zeros((N_NODE, N_NODE), np.float32)
    np.add.at(MT, (f["mp_col"], f["mp_row"]), f["mp_val"])
    MT = MT.astype(bf16)
    featT = np.ascontiguousarray(f["feature"].T).astype(bf16)
    w12 = np.concatenate([f["W1"], f["W2"]], 1).astype(bf16)
    pos_bf = f["pos"].astype(bf16)

    in_maps = []
    for c in range(NCORES):
        rs = slice(c * RPC, (c + 1) * RPC)
        bs = slice(c * BPC, (c + 1) * BPC)
        nb = f["nodes"][bs]
        scT = np.zeros((N_USER, BPC), np.float32)
        scT[nb, np.arange(BPC)] = 1.0
        iid = f["u_iid_list"][nb]                     # [BPC, L]
        ccm = np.zeros((BPC, N_ITEM), np.float32)
        msk = iid != N_ITEM
        rows = np.repeat(np.arange(BPC), L)[msk.ravel()]
        np.add.at(ccm, (rows, iid.ravel()[msk.ravel()]), 1.0)
        user = c < 3
        pre = "ure" if user else "ire"
        ppr = "upr" if user else "ipr"
        m = {
            # pre-tiled [125, 64, 1000]: k-tile t on partition p = row 64p+t
            "featT": np.ascontiguousarray(featT[:, rs]).reshape(KT, NKT, RPC),
            "gT": np.ascontiguousarray(GT[:, rs]).reshape(KT, NKT, RPC),
            "mT": np.ascontiguousarray(MT[:, rs]).reshape(KT, NKT, RPC),
            "w12": w12,
            "pos": np.ascontiguousarray(pos_bf[rs, :]),
            "scT": scT.astype(bf16).reshape(KT, N_USER // KT, BPC),
            "cc": ccm.astype(bf16),
            "prl": np.ascontiguousarray(f["pr_lable"][nb]).astype(np.float32),
            "gw1T_re": np.ascontiguousarray(f[f"g_{pre}_w1"].T).astype(bf16),
            "gw1T_pr": np.ascontiguousarray(f[f"g_{ppr}_w1"].T).astype(bf16),
            "gb1_re": f[f"g_{pre}_b1"].reshape(D, 1).astype(np.float32),
            "gb1_pr": f[f"g_{ppr}_b1"].reshape(D, 1).astype(np.float32),
            "gw2_re": f[f"g_{pre}_w2"].reshape(D, 1).astype(bf16),
            "gw2_pr": f[f"g_{ppr}_w2"].reshape(D, 1).astype(bf16),
            "selscale": (np.array([[1.0 / N_USER], [0.0]], np.float32) if user
                         else np.array([[0.0], [1.0 / N_ITEM]], np.float32)),
            "sel01": (np.array([[1.0], [0.0]], np.float32) if user
                      else np.array([[0.0], [1.0]], np.float32)),
            "rre_row": f["r_re"].reshape(1, D).astype(np.float32),
            "rre_col": f["r_re"].reshape(D, 1).astype(np.float32),
            "rpr_col": f["r_pr"].reshape(D, 1).astype(np.float32),
        }
        in_maps.append(m)
    return in_maps


def finalize(results):
    cols = np.zeros(OUT_COLS, np.float64)
    for c in range(NCORES):
        cols += results[c]["out"].astype(np.float64).sum(0)
    con = cols[C_CON]
    pos_data = (1.0 - NEG_W) * cols[C_A] - 2.0 * cols[C_B]
    pr = cols[C_PR]
    all_data = cols[C_AD]
    loss = NEG_W * all_data + pos_data + PR_W * pr + CON_W * con
    return np.array(loss, dtype=np.float32)


_NC_CACHE = {}


def run_sharded(inputs, trace=False, trace_cores=None):
    from concourse.bass_utils import run_bass_kernel_spmd
    if trace:
        _register_ntff_hook()
    if "nc" not in _NC_CACHE:
        _NC_CACHE["nc"] = build_nc()
    nc = _NC_CACHE["nc"]
    in_maps = prepare_in_maps(inputs)
    kw = {}
    if trace:
        kw = dict(trace=True, trace_cores=trace_cores or [0])
    res = run_bass_kernel_spmd(nc, in_maps, core_ids=list(range(NCORES)), **kw)
    return finalize(res.results), res


def kernel(**inputs) -> np.ndarray:
    loss, _ = run_sharded(inputs, trace=False)
    return loss


def _register_ntff_hook():
    """Optional: register the axon NTFF profiling hook (trace=True support)."""
    if "antenv.axon_hooks" in sys.modules:
        return
    try:
        import importlib.util
        spec = importlib.util.spec_from_file_location(
            "trn_boot", "/root/.axon_site/trn_agent_boot/trn_boot.py")
        trn_boot = importlib.util.module_from_spec(spec)
        spec.loader.exec_module(trn_boot)
        hook = trn_boot._ntff_profile_via_ctypes("/opt/axon/libaxon_pjrt.so")
        mod = types.ModuleType("antenv.axon_hooks")
        mod.get_axon_ntff_profile_hook = lambda: hook
        mod.set_axon_ntff_profile_hook = lambda h: None
        sys.modules["antenv.axon_hooks"] = mod
    except Exception as e:  # profiling is best-effort
        print(f"ntff hook unavailable: {e}", file=sys.stderr)
